# revision 1
# baseline (speedup 1.0000x reference)
"""GCN2Conv (variant=False) Trainium2 kernel.

out = beta * (support @ theta) + (1-beta) * support
support = (1-alpha) * (D^-1/2 (A+I) D^-1/2 @ x) + alpha * h0
beta = log(lamda/l + 1)

Sharding: B=4 graphs over 8 cores -> 2 cores per graph, each owning
m_rows = N/2 = 1500 adjacency rows. x is replicated within a pair, theta
replicated everywhere. The bmm is a local row-block matmul (no cross-device
reduce); only the degree vector (row sums of A+I, needed for the D^-1/2
column scaling of x) is exchanged with a tiny per-pair AllGather.

Device pipeline per core:
  Phase 1 (DMA bound): stream adj row-tiles [128, N]; ACT computes row sums
    via activation(accum_out=...); PE transposes each [128,128] block into
    PSUM; DVE/ACT copy them (cast to bf16) into an SBUF-resident A^T.
  Degree exchange: AllGather [1500] -> [3000] within each pair.
  Phase 2: hi^T = (D x)^T A^T with xs stationary per k-block (N=512 fp32
    PSUM accumulation), fused epilogue in transposed layout, theta matmul,
    transpose back, store.
"""

import math
import sys

import numpy as np

sys.path.insert(0, "/opt/trn_rl_repo")

import concourse.bacc as bacc
import concourse.mybir as mybir
import concourse.tile as tile
from concourse import bass_utils, masks
from concourse.mybir import dt

AF = mybir.ActivationFunctionType

F = 128          # feature dim (= theta size), fixed
P = 128          # SBUF partitions
CHUNK = 512      # phase-2 m-chunk width (one fp32 PSUM bank)

B_FULL, N_FULL = 4, 3000
N_CORES_FULL = 8
M_FULL = N_FULL // 2


def _tile_sizes(total, step):
    return [min(step, total - s) for s in range(0, total, step)]


def build_program(n_nodes, m_rows, n_cores, alpha, beta, at_dtype=dt.bfloat16,
                  debug_dump=False):
    """Build the SPMD Bass program (identical on every core).

    Per-core external inputs (host pre-slices):
      adj_rows [m_rows, n_nodes], x_full [n_nodes, F], x_loc [m_rows, F],
      h0_loc [m_rows, F], theta [F, F].
    Output: out [m_rows, F].
    Cores 2g, 2g+1 own rows [0:m_rows], [m_rows:2*m_rows] of graph g.
    """
    assert n_nodes == 2 * m_rows
    c1 = 1.0 - alpha

    KT = math.ceil(n_nodes / P)        # k blocks (adj cols / nodes)
    kw = _tile_sizes(n_nodes, P)
    MT = math.ceil(m_rows / P)         # local m tiles
    mh = _tile_sizes(m_rows, P)
    mfull, mtail = m_rows // P, m_rows % P
    kfull, ktail = n_nodes // P, n_nodes % P
    # phase-2 chunks: groups of up to 4 full m-tiles (512 cols) or the tail
    # tile alone -- each chunk is one contiguous piece of A^T and gets its
    # own PSUM accumulation bank
    mchunks = []
    ti = 0
    while ti < MT:
        if mh[ti] == P:
            tj = ti
            while tj < MT and mh[tj] == P and tj - ti < 4:
                tj += 1
            mchunks.append((ti * P, (tj - ti) * P, ti, tj, P))
            ti = tj
        else:
            mchunks.append((ti * P, mh[ti], ti, ti + 1, mh[ti]))
            ti += 1

    nc = bacc.Bacc(
        "TRN2", target_bir_lowering=False, debug=False, num_devices=n_cores
    )
    adj = nc.dram_tensor("adj_rows", [m_rows, n_nodes], dt.float32, kind="ExternalInput")
    x_full = nc.dram_tensor("x_full", [n_nodes, F], dt.float32, kind="ExternalInput")
    x_loc = nc.dram_tensor("x_loc", [m_rows, F], dt.float32, kind="ExternalInput")
    h0_loc = nc.dram_tensor("h0_loc", [m_rows, F], dt.float32, kind="ExternalInput")
    theta = nc.dram_tensor("theta", [F, F], dt.float32, kind="ExternalInput")
    out_d = nc.dram_tensor("out", [m_rows, F], dt.float32, kind="ExternalOutput")

    groups = [[2 * g, 2 * g + 1] for g in range(n_cores // 2)]

    with tile.TileContext(nc) as tc:
        from contextlib import ExitStack

        with ExitStack() as ctx:
            ep = ctx.enter_context

            consts = ep(tc.tile_pool(name="consts", bufs=1))
            at_pool = ep(tc.tile_pool(name="at", bufs=1))
            nat_pool = ep(tc.tile_pool(name="nat", bufs=3))
            scr_pool = ep(tc.tile_pool(name="scr", bufs=4))
            deg_pool = ep(tc.tile_pool(name="deg", bufs=1))
            xs_pool = ep(tc.tile_pool(name="xs", bufs=1))
            tvec_pool = ep(tc.tile_pool(name="tvec", bufs=1))
            stream_pool = ep(tc.tile_pool(name="stream", bufs=2))
            sup_pool = ep(tc.tile_pool(name="sup", bufs=2))
            outc_pool = ep(tc.tile_pool(name="outc", bufs=2))
            outt_pool = ep(tc.tile_pool(name="outt", bufs=2))
            ptx_pool = ep(tc.tile_pool(name="ptx", bufs=2, space="PSUM"))
            dram = ep(tc.tile_pool(name="dram", bufs=1, space="DRAM"))

            ident = consts.tile([P, P], dt.float32)
            masks.make_identity(nc, ident[:])

            theta_sb = consts.tile([F, F], dt.float32)
            nc.sync.dma_start(theta_sb[:], theta[:])
            thetaB = consts.tile([F, F], dt.float32)
            nc.vector.tensor_scalar_mul(thetaB[:], theta_sb[:], beta)

            # A^T resident in SBUF: [k_local, (m_tile, kb, m_local)] -- one
            # contiguous [KT, 128] region per m-tile so the blocked xbar
            # transpose writes it in a single instruction
            AT = at_pool.tile([P, MT * KT * P], at_dtype)
            AT4 = AT[:].rearrange("p (i kb m) -> p i kb m", i=MT, kb=KT)

            # local degree accumulator: col i = row sums of local m-tile i
            deg_sb = deg_pool.tile([P, MT], dt.float32)
            nc.gpsimd.memset(deg_sb[:], 1.0)  # garbage lanes stay rsqrt-safe

            # ---------------- Phase 1: stream adj, rowsum + transpose ----------
            # SWDGE plain fp32 half-tile loads; ACT casts to bf16 + row-sums
            # (two halves, accumulators summed later); one blocked xbar
            # transpose per m-tile from the bf16 tile into A^T.
            from concourse.tile import add_dep_helper as _adh

            deg_sbB = deg_pool.tile([P, MT], dt.float32, tag="degB")
            nc.gpsimd.memset(deg_sbB[:], 1.0)
            HALF = KT * P // 2  # columns per load half (KT is even or padded)
            half_w = [min(HALF, n_nodes), max(0, n_nodes - HALF)]

            # The xbar-transpose's data accesses are invisible to Tile's dep
            # tracker, so fence manually:
            #  - RAW: transpose waits the two ACT cast+rowsum ops that write
            #    its bf16 source tile
            #  - WAR: the ACTs reusing a bf16 slot depend on the transpose
            #    that last read it (HWDGE producer -> waits its DMA lane)
            t_insts = []
            NAT16_BUFS = 3
            for i in range(MT):
                h = mh[i]
                nat16 = nat_pool.tile([P, KT * P], at_dtype, tag="nat16")
                acts = []
                for hf in range(2):
                    wcol = half_w[hf]
                    natf = scr_pool.tile([P, HALF], dt.float32, tag="natf")
                    nc.sync.dma_start(
                        natf[:h, 0:wcol],
                        adj[P * i : P * i + h, hf * HALF : hf * HALF + wcol],
                    )
                    dst = deg_sb if hf == 0 else deg_sbB
                    act = nc.scalar.activation(
                        nat16[:h, hf * HALF : hf * HALF + wcol],
                        natf[:h, 0:wcol],
                        AF.Copy,
                        accum_out=dst[:h, i : i + 1],
                    )
                    if i >= NAT16_BUFS:
                        _adh(act.ins, t_insts[i - NAT16_BUFS].ins, sync=True,
                             reason="nat16 slot WAR vs xbar transpose")
                    acts.append(act)

                # one blocked transpose for the whole row-tile:
                # in [128, KT*128] -> out [128, KT, 128] (3D out folds kb into
                # the logical partition dim; out region contiguous). Tail
                # tiles read/write garbage rows beyond h -- never consumed.
                t_inst = nc.sync.dma_start_transpose(
                    AT4[:, i, :, :],
                    nat16[:P, 0 : KT * P],
                )
                for act in acts:
                    _adh(t_inst.ins, act.ins, sync=True,
                         reason="xbar transpose RAW fence via ACT cast")
                t_insts.append(t_inst)

            # ---------------- degree: +1 self loop, pair exchange --------------
            degp = deg_pool.tile([P, MT], dt.float32)
            nc.vector.tensor_add(degp[:], deg_sb[:], deg_sbB[:])
            nc.vector.tensor_scalar_add(degp[:], degp[:], 1.0)

            degT_ps = ptx_pool.tile([P, P], dt.float32, tag="sm")
            nc.tensor.transpose(degT_ps[:MT, :P], degp[:P, :MT], ident[:P, :P])
            degT = deg_pool.tile([MT, P], dt.float32)
            nc.vector.tensor_copy(degT[:], degT_ps[:MT, :P])

            deg_loc_d = dram.tile([m_rows], dt.float32)
            deg_full_d = dram.tile([n_nodes], dt.float32)
            if mfull:
                nc.gpsimd.dma_start(
                    deg_loc_d[0 : mfull * P].rearrange("(a b) -> a b", b=P),
                    degT[0:mfull, :],
                )
            if mtail:
                nc.gpsimd.dma_start(
                    deg_loc_d[mfull * P : m_rows].rearrange("(a b) -> a b", a=1),
                    degT[mfull : mfull + 1, 0:mtail],
                )
            # xbar-mode transposes must not run concurrently with the
            # collective's DMAs (HW deadlock) and phase 2 must see completed
            # A^T -- gate on the transpose-completion semaphore.
            ag = nc.gpsimd.collective_compute(
                "AllGather",
                mybir.AluOpType.bypass,
                replica_groups=groups,
                ins=[deg_loc_d[:]],
                outs=[deg_full_d[:]],
            )
            _adh(ag.ins, t_insts[-1].ins, sync=True,
                 reason="xbar-vs-collective serialization")

            # PE HAM warm-up: dummy matmuls right after the AllGather so the
            # phase-2 matmuls start at the warm 2.4 GHz clock
            with tc.tile_pool(name="warm_ps", bufs=1, space="PSUM") as warm_pool:
                wp = warm_pool.tile([P, CHUNK], dt.float32)
                n_warm = 18
                for j in range(n_warm):
                    wmm = nc.tensor.matmul(
                        wp[:P, 0:CHUNK],
                        AT[:P, 0:P],
                        AT[:P, 0:CHUNK],
                        start=(j == 0),
                        stop=(j == n_warm - 1),
                    )
                    if j == 0:
                        warm0 = wmm

            # local row-scale vector in free-aligned layout; read back from the
            # DRAM copy (avoids SBUF->SBUF DMA, which deadlocks vs xbar mode)
            vecs = tvec_pool.tile([P, m_rows], dt.float32)
            deg_row = vecs[0:1, :]
            nc.gpsimd.dma_start(
                deg_row[0:1, 0:m_rows],
                deg_loc_d[:].rearrange("(a b) -> a b", a=1),
            )
            dis_row = vecs[0:1, :]
            nc.vector.reciprocal(dis_row, deg_row)
            nc.scalar.sqrt(dis_row, dis_row)

            # broadcast dis across partitions, then rs = c1*dis, s1 = c1*dis^2
            s1_b = tvec_pool.tile([P, m_rows], dt.float32, tag="s1_b")
            nc.gpsimd.partition_broadcast(s1_b[:], dis_row)
            rs_b = tvec_pool.tile([P, m_rows], dt.float32, tag="rs_b")
            nc.vector.tensor_scalar_mul(rs_b[:], s1_b[:], c1)
            nc.vector.tensor_mul(s1_b[:], s1_b[:], rs_b[:])

            # global degrees -> dis per k-block [P, KT]
            dgT = deg_pool.tile([P, P], dt.float32, tag="dgT")
            nc.gpsimd.memset(dgT[:KT, :], 1.0)
            dg_lds = []
            if kfull:
                dg_lds.append(nc.gpsimd.dma_start(
                    dgT[0:kfull, 0:P],
                    deg_full_d[0 : kfull * P].rearrange("(a b) -> a b", b=P),
                ))
            if ktail:
                dg_lds.append(nc.gpsimd.dma_start(
                    dgT[kfull : kfull + 1, 0:ktail],
                    deg_full_d[kfull * P : n_nodes],
                ))
            # anchor the PE warm-up on the first post-AG data load so the
            # warm clock carries into the phase-2 matmuls
            for dl in dg_lds[:1]:
                _adh(warm0.ins, dl.ins, sync=True, reason="warmup after AG data")
            dg_ps = ptx_pool.tile([P, P], dt.float32, tag="sm")
            nc.tensor.transpose(dg_ps[:P, :KT], dgT[:KT, :P], ident[:KT, :KT])
            disg = deg_pool.tile([P, KT], dt.float32)
            nc.vector.tensor_copy(disg[:], dg_ps[:P, :KT])
            nc.vector.reciprocal(disg[:], disg[:])
            nc.scalar.sqrt(disg[:], disg[:])

            # xs = D^-1/2 x in [k_local, (kb, f)] layout, cast to at_dtype
            xg = xs_pool.tile([P, KT * F], at_dtype)
            if kfull:
                nc.gpsimd.dma_start(
                    xg[:].rearrange("p (kb f) -> p kb f", kb=KT)[:, 0:kfull, :],
                    x_full[0 : kfull * P, :].rearrange("(kb p) f -> p kb f", p=P),
                )
            if ktail:
                nc.gpsimd.dma_start(
                    xg[0:ktail, kfull * F : (kfull + 1) * F],
                    x_full[kfull * P : n_nodes, :],
                )
            xs = xs_pool.tile([P, KT * F], at_dtype)
            for kb in range(KT):
                w = kw[kb]
                nc.vector.tensor_scalar_mul(
                    xs[:w, kb * F : kb * F + F],
                    xg[:w, kb * F : kb * F + F],
                    disg[:w, kb : kb + 1],
                )

            # x_loc / h0_loc transposed: xT [f, m], h0aT = alpha * h0^T
            # (bulk SWDGE loads in the per-m-tile [p, (i f)] layout)
            xT = xs_pool.tile([P, m_rows], dt.float32, tag="xT")
            h0aT = xs_pool.tile([P, m_rows], dt.float32, tag="h0aT")
            xn_all = xs_pool.tile([P, MT * F], dt.float32, tag="xn_all")
            hn_all = xs_pool.tile([P, MT * F], dt.float32, tag="hn_all")
            for src, dst in ((x_loc, xn_all), (h0_loc, hn_all)):
                if mfull:
                    nc.gpsimd.dma_start(
                        dst[:].rearrange("p (i f) -> p i f", i=MT)[:, 0:mfull, :],
                        src[0 : mfull * P, :].rearrange("(i p) f -> p i f", p=P),
                    )
                if mtail:
                    nc.gpsimd.dma_start(
                        dst[0:mtail, mfull * F : (mfull + 1) * F],
                        src[mfull * P : m_rows, :],
                    )
            for i in range(MT):
                h = mh[i]
                xt_ps = ptx_pool.tile([P, P], dt.float32, tag="sm")
                nc.tensor.transpose(
                    xt_ps[:F, :h], xn_all[:h, i * F : i * F + F], ident[:h, :h]
                )
                nc.vector.tensor_copy(xT[:, P * i : P * i + h], xt_ps[:F, :h])

                ht_ps = ptx_pool.tile([P, P], dt.float32, tag="sm")
                nc.tensor.transpose(
                    ht_ps[:F, :h], hn_all[:h, i * F : i * F + F], ident[:h, :h]
                )
                nc.scalar.activation(
                    h0aT[:, P * i : P * i + h], ht_ps[:F, :h], AF.Copy, scale=alpha
                )

            # qT = s1 * x^T + alpha * h0^T  (everything but the hi term)
            qT = xs_pool.tile([P, m_rows], dt.float32, tag="qT")
            nc.vector.tensor_mul(qT[:], xT[:], s1_b[:])
            nc.vector.tensor_add(qT[:], qT[:], h0aT[:])

            if debug_dump:
                dbg_at = nc.dram_tensor(
                    "dbg_at", [P, MT * KT * P], at_dtype, kind="ExternalOutput"
                )
                dbg_dma = nc.sync.dma_start(dbg_at[:], AT[:])
                _adh(dbg_dma.ins, t_insts[-1].ins, sync=True, reason="dbg")
                dbg_xs = nc.dram_tensor(
                    "dbg_xs", [P, KT * F], at_dtype, kind="ExternalOutput"
                )
                nc.sync.dma_start(dbg_xs[:], xs[:])

            # ---------------- Phase 2: matmuls + epilogue ----------------------
            with tc.tile_pool(name="hi_ps", bufs=2, space="PSUM") as hi_pool, \
                 tc.tile_pool(name="o2_ps", bufs=2, space="PSUM") as o2_pool:
                for s, wc, ia, ib, tw in mchunks:
                    hiT = hi_pool.tile([P, CHUNK], dt.float32)
                    for kb in range(KT):
                        w = kw[kb]
                        nc.tensor.matmul(
                            hiT[:F, 0:wc],
                            xs[:w, kb * F : kb * F + F],
                            AT4[:w, ia:ib, kb, 0:tw],
                            start=(kb == 0),
                            stop=(kb == KT - 1),
                        )
                    supT = sup_pool.tile([P, CHUNK], dt.float32)
                    nc.vector.tensor_mul(supT[:, 0:wc], hiT[:F, 0:wc], rs_b[:, s : s + wc])
                    nc.vector.tensor_add(supT[:, 0:wc], supT[:, 0:wc], qT[:, s : s + wc])

                    o2T = o2_pool.tile([P, CHUNK], dt.float32)
                    nc.tensor.matmul(
                        o2T[:F, 0:wc], thetaB[:F, :F], supT[:F, 0:wc],
                        start=True, stop=True,
                    )
                    outT = outc_pool.tile([P, CHUNK], dt.float32)
                    nc.vector.scalar_tensor_tensor(
                        outT[:, 0:wc], supT[:, 0:wc], 1.0 - beta, o2T[:F, 0:wc],
                        mybir.AluOpType.mult, mybir.AluOpType.add,
                    )

                    # back to natural [m, f] and store
                    for off in range(0, wc, P):
                        hh = min(P, wc - off)
                        ot_ps = ptx_pool.tile([P, P], dt.float32, tag="sm")
                        nc.tensor.transpose(
                            ot_ps[:hh, :F], outT[:F, off : off + hh], ident[:F, :F]
                        )
                        ot = outt_pool.tile([P, F], dt.float32)
                        nc.vector.tensor_copy(ot[:hh, :], ot_ps[:hh, :F])
                        nc.sync.dma_start(
                            out_d[s + off : s + off + hh, :], ot[:hh, :]
                        )

    nc.compile()
    return nc


def _copy(eng, out_ap, in_ap):
    if hasattr(eng, "tensor_copy"):
        eng.tensor_copy(out_ap, in_ap)
    else:
        eng.copy(out_ap, in_ap)


def make_in_maps(x, adj, h0, theta, n_cores):
    m = x.shape[1] // 2
    in_maps = []
    for c in range(n_cores):
        b, half = c // 2, c % 2
        r0 = half * m
        in_maps.append(
            {
                "adj_rows": adj[b, r0 : r0 + m, :],
                "x_full": x[b],
                "x_loc": x[b, r0 : r0 + m, :],
                "h0_loc": h0[b, r0 : r0 + m, :],
                "theta": theta,
            }
        )
    return in_maps


_CACHE = {}


def _get_program(key, *args, **kwargs):
    if key not in _CACHE:
        _CACHE[key] = build_program(*args, **kwargs)
    return _CACHE[key]


def kernel(x, adj, h0, theta, lamda, alpha, l):
    x = np.asarray(x, dtype=np.float32)
    adj = np.asarray(adj, dtype=np.float32)
    h0 = np.asarray(h0, dtype=np.float32)
    theta = np.asarray(theta, dtype=np.float32)
    lamda_f = float(np.asarray(lamda))
    alpha_f = float(np.asarray(alpha))
    l_f = float(np.asarray(l))
    beta_f = float(math.log(lamda_f / l_f + 1.0))

    B, N, Fdim = x.shape
    assert (B, N, Fdim) == (B_FULL, N_FULL, F)
    M = N // 2

    nc = _get_program(
        ("full", alpha_f, beta_f), N, M, N_CORES_FULL, alpha_f, beta_f
    )

    in_maps = make_in_maps(x, adj, h0, theta, N_CORES_FULL)
    res = bass_utils.run_bass_kernel_spmd(
        nc, in_maps, list(range(N_CORES_FULL))
    ).results

    out = np.empty((B, N, Fdim), dtype=np.float32)
    for c in range(N_CORES_FULL):
        b, half = c // 2, c % 2
        out[b, half * M : (half + 1) * M, :] = res[c]["out"]
    return out

